# revision 16
# baseline (speedup 1.0000x reference)
"""MBConv block with MoE routing (depthwise + pointwise expert kernels) on 8 trn2 cores.

Sharding: pure data parallel — batch 64 split 8 samples per core; all weights
replicated. Device kernel computes routing, expert-weight aggregation, expand
conv, per-sample depthwise conv (diagonal-matmul formulation on TensorE),
squeeze-excitation, pointwise projection, BN folds and residual.

Self-contained: hardcodes all shapes; host side only reshapes/prepacks weights.
"""

import os
import sys
import time

for _p in ("/opt/trn_rl_repo", os.path.expanduser("~/.axon_site/_ro/trn_rl_repo")):
    if os.path.isdir(_p) and _p not in sys.path:
        sys.path.insert(0, _p)

import contextlib

import numpy as np

import concourse.bacc as bacc
import concourse.bass as bass
import concourse.tile as tile
from concourse import mybir

F32 = mybir.dt.float32
BF16 = mybir.dt.bfloat16
AF = mybir.ActivationFunctionType
ALU = mybir.AluOpType
AX = mybir.AxisListType

# dims (must match the problem spec)
B, CIN, H, W = 64, 96, 28, 28
NCORES = 8
BL = B // NCORES          # 8 samples per core
E = 4
HID = 576
KK = 5
T = KK * KK               # 25 taps
RED = 24                  # SE reduced dim
RHID = 24                 # routing hidden
COUT = 96
EPS = 1e-3
HW = H * W                # 784
NG = 5                    # ceil(576/128) channel chunks
GP = 128
HIDP = NG * GP            # 640 padded
PW = 32                   # padded row stride
PH = 32                   # padded rows
NH = 2                    # output row halves (14 rows each)
RH = H // NH              # 14
NF = RH * W               # 392 free elems per half


def _build_program(reps=1, ablate=()):
    nc = bacc.Bacc(None, target_bir_lowering=False)

    dt = lambda name, shape: nc.dram_tensor(name, shape, F32, kind="ExternalInput")
    x_d = dt("x", [CIN, BL, HW])
    xbf_d = nc.dram_tensor("xbf", [CIN, BL, HW], BF16, kind="ExternalInput")
    expbf_d = nc.dram_tensor("expbf", [CIN, HIDP], BF16, kind="ExternalInput")
    identbf_d = nc.dram_tensor("identbf", [GP, GP], BF16, kind="ExternalInput")
    expT_d = dt("expT", [CIN, HIDP])
    a1_d = dt("a1", [GP, NG])
    b1_d = dt("b1", [GP, NG])
    a2_d = dt("a2", [GP, NG])
    b2_d = dt("b2", [GP, NG])
    a3_d = dt("a3", [COUT, 1])
    b3_d = dt("b3", [COUT, 1])
    dwT_d = dt("dwT", [GP, E, NG, T])
    pwT_d = dt("pwT", [GP, E, NG, COUT])
    sw1_d = dt("sw1", [GP, NG, RED])
    sw2b_d = dt("sw2b", [RED, NG, GP])
    b2se_d = dt("b2se", [GP, NG])
    rw1_d = dt("rw1", [CIN, RHID])
    rb1_d = dt("rb1", [RHID, 1])
    rw2_d = dt("rw2", [RHID, E])
    rb2_d = dt("rb2", [BL, E])
    sb1_d = dt("sb1", [RED, 1])
    ident_d = dt("ident", [GP, GP])
    y_d = nc.dram_tensor("y", [BL, COUT, HW], F32, kind="ExternalOutput")

    with tile.TileContext(nc) as tc:
        with (
            tc.tile_pool(name="consts", bufs=1) as cp,
            tc.tile_pool(name="dram", bufs=1, space="DRAM") as dp,
            tc.tile_pool(name="xpad", bufs=1) as xpp,
            tc.tile_pool(name="out2", bufs=1) as o2p,
            tc.tile_pool(name="diag", bufs=3) as dgp,
            tc.tile_pool(name="cacc", bufs=2) as accp,
            tc.tile_pool(name="wscp", bufs=2) as wsp,
            tc.tile_pool(name="outb", bufs=2) as obp,
            tc.tile_pool(name="small", bufs=2) as smp,
            tc.tile_pool(name="ppex", bufs=2, space="PSUM") as ppex,
            tc.tile_pool(name="pdw", bufs=2, space="PSUM") as pdwp,
            tc.tile_pool(name="ppw", bufs=1, space="PSUM") as ppwp,
        ):
            # ---- persistent consts ----
            x_sb = cp.tile([CIN, BL, HW], F32, tag="x_sb")
            x_bf = cp.tile([CIN, BL, HW], BF16, tag="x_bf")
            expT = cp.tile([CIN, HIDP], BF16, tag="expT")
            a1 = cp.tile([GP, NG], F32, tag="a1")
            b1 = cp.tile([GP, NG], F32, tag="b1")
            a2 = cp.tile([GP, NG], F32, tag="a2")
            b2 = cp.tile([GP, NG], F32, tag="b2")
            a3 = cp.tile([COUT, 1], F32, tag="a3")
            b3 = cp.tile([COUT, 1], F32, tag="b3")
            dwT = cp.tile([GP, E, NG, T], F32, tag="dwT")
            pwT = cp.tile([GP, E, NG, COUT], F32, tag="pwT")
            sw1 = cp.tile([GP, NG, RED], F32, tag="sw1")
            sw2b = cp.tile([RED, NG, GP], F32, tag="sw2b")
            b2se = cp.tile([GP, NG], F32, tag="b2se")
            rw1 = cp.tile([CIN, RHID], F32, tag="rw1")
            rb1 = cp.tile([RHID, 1], F32, tag="rb1")
            rw2 = cp.tile([RHID, E], F32, tag="rw2")
            rb2 = cp.tile([BL, E], F32, tag="rb2")
            sb1 = cp.tile([RED, 1], F32, tag="sb1")
            ident = cp.tile([GP, GP], BF16, tag="ident")
            kern = cp.tile([GP, NG, BL, T], F32, tag="kern")
            kern_bf = cp.tile([GP, NG, BL, T], BF16, tag="kern_bf")
            pwag = cp.tile([GP, BL, NG, COUT], F32, tag="pwag")
            rw_bc = cp.tile([GP, BL * E], F32, tag="rw_bc")

            # ---- padded x1 tiles: 2 buffers x NG chunks, zeroed once ----
            xp_t = [
                [xpp.tile([GP, 1028], BF16, tag=f"xp{s}g{g}", name=f"xp{s}g{g}")
                 for g in range(NG)]
                for s in range(2)
            ]
            for s in range(2):
                for g in range(NG):
                    nc.gpsimd.memset(xp_t[s][g][:], 0.0)

            out2_t = [o2p.tile([GP, NG, HW], BF16, tag=f"o2{s}", name=f"o2{s}")
                      for s in range(2)]
            s_parts = [cp.tile([GP, NG], F32, tag=f"sp{b}", name=f"sp{b}")
                       for b in range(BL)]
            ident_ap = ident[:]

            def build_diag(eng, dg, b, g):
                # dg[:, t, :] = diag(kern[:, g, b, t]) ; one fused op:
                # out = ident(bcast over taps) * kern(bcast over cols)
                i_b = bass.AP(tensor=ident_ap.tensor, offset=ident_ap.offset,
                              ap=[ident_ap.ap[0], [0, T], [1, GP]])
                k_ap = kern_bf[:, g, b, :]
                k_b = bass.AP(tensor=k_ap.tensor, offset=k_ap.offset,
                              ap=[k_ap.ap[0], k_ap.ap[-1], [0, GP]])
                eng.tensor_mul(dg[:], i_b, k_b)

            def emit_se_pw(b, slot):
                """SE chain + pointwise matmul + bn3/residual + store for sample b."""
                s_sum = s_parts[b]
                pz = ppex.tile([RED, 1], F32, tag="pex", name="pz")
                for g in range(NG):
                    nc.tensor.matmul(pz[:], sw1[:, g], s_sum[:, g : g + 1],
                                     start=(g == 0), stop=(g == NG - 1))
                zt = smp.tile([RED, 1], F32, tag="zt", name="zt")
                nc.scalar.activation(zt[:], pz[:], AF.Silu, bias=sb1[:], scale=1.0)
                psc = ppex.tile([GP, NG], F32, tag="pex", name="psc")
                for g in range(NG):
                    nc.tensor.matmul(psc[:, g : g + 1], sw2b[:, g], zt[:],
                                     start=True, stop=True)
                # sigmoid via tanh (stays in the silu ACT table set):
                # sigmoid(p + b) = 0.5 + 0.5*tanh(0.5*p + 0.5*b); b2se is pre-halved
                sc = smp.tile([GP, NG], F32, tag="sc", name="sc")
                for g in range(NG):
                    nc.scalar.activation(sc[:, g : g + 1], psc[:, g : g + 1],
                                         AF.Tanh, bias=b2se[:, g : g + 1],
                                         scale=0.5)
                nc.vector.tensor_scalar(sc[:], sc[:], 0.5, 0.5,
                                        op0=ALU.mult, op1=ALU.add)
                wsc = wsp.tile([GP, NG, COUT], BF16, tag="wsc", name="wsc")
                for g in range(NG):
                    nc.vector.tensor_scalar_mul(wsc[:, g], pwag[:, b, g],
                                                sc[:, g : g + 1])
                # pointwise projection, accumulate over channel chunks
                po = ppwp.tile([COUT, NH, 512], F32, tag="po", name="po")
                if "nopw" not in ablate:
                    for nh in range(NH):
                        for g in range(NG):
                            nc.tensor.matmul(
                                po[:, nh, :NF], wsc[:, g],
                                out2_t[slot][:, g, nh * NF : (nh + 1) * NF],
                                start=(g == 0), stop=(g == NG - 1))
                ob = obp.tile([COUT, HW], F32, tag="ob", name="ob")
                for nh in range(NH):
                    sl = slice(nh * NF, (nh + 1) * NF)
                    if "nopw" in ablate:
                        nc.vector.tensor_copy(ob[:, sl], x_sb[:, b, sl])
                        continue
                    nc.vector.scalar_tensor_tensor(
                        ob[:, sl], po[:, nh, :NF], a3[:], x_sb[:, b, sl],
                        op0=ALU.mult, op1=ALU.add)
                    nc.vector.tensor_scalar_add(ob[:, sl], ob[:, sl], b3[:])
                nc.sync.dma_start(y_d[b], ob[:])

            def emit_body():
                nc.sync.dma_start(x_sb[:], x_d[:])
                nc.sync.dma_start(x_bf[:], xbf_d[:])
                for t_sb, t_d in [
                    (expT, expbf_d), (a1, a1_d), (b1, b1_d), (a2, a2_d),
                    (b2, b2_d), (a3, a3_d), (b3, b3_d), (dwT, dwT_d),
                    (pwT, pwT_d), (sw1, sw1_d), (sw2b, sw2b_d), (b2se, b2se_d),
                    (rw1, rw1_d), (rb1, rb1_d), (rw2, rw2_d), (rb2, rb2_d),
                    (sb1, sb1_d), (ident, identbf_d),
                ]:
                    nc.sync.dma_start(t_sb[:], t_d[:])

                # routing: pool -> MLP -> softmax (transposed: samples on partitions)
                xsum = cp.tile([CIN, BL], F32, tag="xsum", name="xsum")
                nc.vector.tensor_reduce(xsum[:], x_sb[:], axis=AX.X, op=ALU.add)
                ph1 = ppex.tile([RHID, BL], F32, tag="pex", name="ph1")
                nc.tensor.matmul(ph1[:], rw1[:], xsum[:], start=True, stop=True)
                hdn = cp.tile([RHID, BL], F32, tag="hdn", name="hdn")
                nc.scalar.activation(hdn[:], ph1[:], AF.Relu, bias=rb1[:], scale=1.0)
                pl2 = ppex.tile([BL, E], F32, tag="pex", name="pl2")
                nc.tensor.matmul(pl2[:], hdn[:], rw2[:], start=True, stop=True)
                lt = cp.tile([BL, E], F32, tag="lt", name="lt")
                nc.vector.tensor_add(lt[:], pl2[:], rb2[:])
                mx = cp.tile([BL, 1], F32, tag="mx", name="mx")
                nc.vector.reduce_max(mx[:], lt[:], axis=AX.X)
                nc.vector.tensor_scalar_sub(lt[:], lt[:], mx[:])
                el = cp.tile([BL, E], F32, tag="el", name="el")
                nc.scalar.activation(el[:], lt[:], AF.Exp)
                es = cp.tile([BL, 1], F32, tag="es", name="es")
                nc.vector.reduce_sum(es[:], el[:], axis=AX.X)
                einv = cp.tile([BL, 1], F32, tag="einv", name="einv")
                nc.vector.reciprocal(einv[:], es[:])
                rwT = cp.tile([BL, E], F32, tag="rwT", name="rwT")
                nc.vector.tensor_scalar_mul(rwT[:], el[:], einv[:])
                # broadcast rw to all 128 partitions via DRAM bounce
                rw_dram = dp.tile([BL, E], F32, tag="rwd", name="rwd")
                nc.sync.dma_start(rw_dram[:], rwT[:])
                rwd_ap = rw_dram[:]
                bcast_src = bass.AP(
                    tensor=rwd_ap.tensor, offset=rwd_ap.offset,
                    ap=[[0, GP], [1, BL * E]],
                )
                nc.sync.dma_start(rw_bc[:], bcast_src)

                # expert-weight aggregation (runtime routing weights)
                for b in range(BL):
                    kv = kern[:, :, b, :]
                    pv = pwag[:, b]
                    for e in range(E):
                        s_ap = rw_bc[:, E * b + e : E * b + e + 1]
                        if e == 0:
                            nc.vector.tensor_scalar_mul(kv, dwT[:, e], s_ap)
                            nc.vector.tensor_scalar_mul(pv, pwT[:, e], s_ap)
                        else:
                            nc.vector.scalar_tensor_tensor(
                                kv, dwT[:, e], s_ap, kv, op0=ALU.mult, op1=ALU.add)
                            nc.vector.scalar_tensor_tensor(
                                pv, pwT[:, e], s_ap, pv, op0=ALU.mult, op1=ALU.add)

                nc.vector.tensor_copy(kern_bf[:], kern[:])

                for b in range(BL):
                    slot = b % 2
                    if b > 0:
                        emit_se_pw(b - 1, 1 - slot)
                    # expand conv + bn1 + silu into padded layout
                    for g in range(NG):
                        for nh in range(NH):
                            pex = ppex.tile([GP, NF], F32, tag="pex", name="pex")
                            nc.tensor.matmul(
                                pex[:], expT[:, g * GP : (g + 1) * GP],
                                x_bf[:, b, nh * NF : (nh + 1) * NF],
                                start=True, stop=True)
                            xpv = xp_t[slot][g][:, 68 + nh * 448 : 68 + nh * 448 + 434]
                            xpo = bass.AP(tensor=xpv.tensor, offset=xpv.offset,
                                          ap=[xpv.ap[0], [PW, RH], [1, W]])
                            nc.scalar.activation(
                                xpo, pex[:], AF.Silu,
                                bias=b1[:, g : g + 1], scale=a1[:, g : g + 1])
                    # diagonal weight matrices for this sample
                    dgs = []
                    if "nodiag" not in ablate:
                        for g in range(NG - 1):
                            dg = dgp.tile([GP, T, GP], BF16, tag="dg", name="dg")
                            build_diag(nc.gpsimd, dg, b, g)
                            dgs.append(dg)
                    # depthwise conv: 25 accumulated diag matmuls per chunk/half
                    for g in range(NG - 1):
                        pdw = pdwp.tile([GP, NH, 512], F32, tag="pdw", name="pdw")
                        taps = [] if "noconv" in ablate else list(range(T))
                        if "conv1tap" in ablate:
                            taps = [12]
                        for nh in range(NH):
                            for ti, t in enumerate(taps):
                                kh, kw = divmod(t, KK)
                                lhs = (ident[:, :] if "nodiag" in ablate
                                       else dgs[g][:, t, :])
                                off = 448 * nh + 32 * kh + kw
                                rhs = xp_t[slot][g][:, off : off + 448]
                                nc.tensor.matmul(pdw[:, nh, :448], lhs, rhs,
                                                 start=(ti == 0),
                                                 stop=(ti == len(taps) - 1))
                        # bn2 + silu over both banks in one op (+ SE spatial sum)
                        pv = pdw[:, 0, 2:3]
                        pvo = bass.AP(tensor=pv.tensor, offset=pv.offset,
                                      ap=[pv.ap[0], [512, NH], [PW, RH], [1, W]])
                        nc.scalar.activation(
                            out2_t[slot][:, g, :], pvo, AF.Silu,
                            bias=b2[:, g : g + 1], scale=a2[:, g : g + 1],
                            accum_out=s_parts[b][:, g : g + 1])
                    # ---- g=4 chunk on DVE: bf16 scalar*tensor+acc tap chain ----
                    g4 = NG - 1
                    acc = accp.tile([GP, 896], BF16, tag="acc", name="acc")
                    taps4 = [12] if "conv1tap" in ablate else list(range(T))
                    for ti, t in enumerate(taps4):
                        kh, kw = divmod(t, KK)
                        xin = xp_t[slot][g4][:, 32 * kh + kw : 32 * kh + kw + 896]
                        k_ap = kern[:, g4, b, t : t + 1]
                        if ti == 0:
                            nc.vector.tensor_scalar_mul(acc[:], xin, k_ap)
                        else:
                            nc.vector.scalar_tensor_tensor(
                                acc[:], xin, k_ap, acc[:],
                                op0=ALU.mult, op1=ALU.add)
                    av = acc[:, 2:868]
                    avo = bass.AP(tensor=av.tensor, offset=av.offset,
                                  ap=[av.ap[0], [PW, H], [1, W]])
                    nc.scalar.activation(
                        out2_t[slot][:, g4, :], avo, AF.Silu,
                        bias=b2[:, g4 : g4 + 1], scale=a2[:, g4 : g4 + 1],
                        accum_out=s_parts[b][:, 4:5])
                emit_se_pw(BL - 1, (BL - 1) % 2)

            loop_ctx = (tc.For_i(0, reps, 1, hint_engines=(mybir.EngineType.PE,))
                        if reps > 1 else contextlib.nullcontext())
            with loop_ctx:
                emit_body()

    nc.compile()
    return nc


_NC = None


def _get_nc():
    global _NC
    if _NC is None:
        _NC = _build_program()
    return _NC


def _prep_maps(x, r_w1, r_b1, r_w2, r_b2, exp_w,
               bn1_g, bn1_b, bn1_m, bn1_v, dw_w,
               bn2_g, bn2_b, bn2_m, bn2_v,
               se_w1, se_b1, se_w2, se_b2, pw_w,
               bn3_g, bn3_b, bn3_m, bn3_v):
    f = np.float32
    bfdt = mybir.dt.np(BF16)
    x = np.asarray(x, f).reshape(B, CIN, HW)

    def fold_bn(g, bvec, m, v):
        a = np.asarray(g, f) / np.sqrt(np.asarray(v, f) + EPS)
        return a, np.asarray(bvec, f) - np.asarray(m, f) * a

    a1v, b1v = fold_bn(bn1_g, bn1_b, bn1_m, bn1_v)
    a2v, b2v = fold_bn(bn2_g, bn2_b, bn2_m, bn2_v)
    a3v, b3v = fold_bn(bn3_g, bn3_b, bn3_m, bn3_v)

    def chunk(v):  # [HID] -> [GP, NG] padded
        vp = np.concatenate([np.asarray(v, f), np.zeros(HIDP - HID, f)])
        return vp.reshape(NG, GP).T.copy()

    expT = np.zeros((CIN, HIDP), f)
    expT[:, :HID] = np.asarray(exp_w, f).T
    dwf = np.asarray(dw_w, f).reshape(E, HID, T)
    dwT = np.zeros((GP, E, NG, T), f)
    pwT = np.zeros((GP, E, NG, COUT), f)
    sw1 = np.zeros((GP, NG, RED), f)
    sw2b = np.zeros((RED, NG, GP), f)
    b2se = np.zeros((GP, NG), f)
    for g in range(NG):
        n = min(GP, HID - g * GP)
        cs = slice(g * GP, g * GP + n)
        dwT[:n, :, g, :] = dwf[:, cs, :].transpose(1, 0, 2)
        pwT[:n, :, g, :] = np.asarray(pw_w, f)[:, :, cs].transpose(2, 0, 1)
        sw1[:n, g, :] = (np.asarray(se_w1, f)[:, cs] / HW).T
        sw2b[:, g, :n] = np.asarray(se_w2, f)[cs, :].T
        b2se[:n, g] = np.asarray(se_b2, f)[cs] / 2

    common = dict(
        expT=expT, a1=chunk(a1v), b1=chunk(b1v), a2=chunk(a2v), b2=chunk(b2v),
        a3=a3v.reshape(COUT, 1), b3=b3v.reshape(COUT, 1),
        dwT=dwT, pwT=pwT, sw1=sw1, sw2b=sw2b, b2se=b2se,
        rw1=(np.asarray(r_w1, f).T / HW).copy(),
        rb1=np.asarray(r_b1, f).reshape(RHID, 1),
        rw2=np.asarray(r_w2, f).T.copy(),
        rb2=np.tile(np.asarray(r_b2, f), (BL, 1)),
        sb1=np.asarray(se_b1, f).reshape(RED, 1),
        ident=np.eye(GP, dtype=f),
        expbf=expT.astype(bfdt),
        identbf=np.eye(GP, dtype=bfdt),
    )
    out = []
    for c in range(NCORES):
        xs = np.ascontiguousarray(x[c * BL : (c + 1) * BL].transpose(1, 0, 2))
        out.append(dict(common, x=xs, xbf=xs.astype(bfdt)))
    return out


def kernel(**inputs):
    from concourse.bass_utils import run_bass_kernel_spmd

    nc = _get_nc()
    in_maps = _prep_maps(**inputs)
    res = run_bass_kernel_spmd(nc, in_maps, core_ids=list(range(NCORES)))
    y = np.concatenate([res.results[c]["y"] for c in range(NCORES)], axis=0)
    return y.reshape(B, COUT, H, W).astype(np.float32)


if __name__ == "__main__":
    t0 = time.time()
    nc = _get_nc()
    print(f"build+compile: {time.time()-t0:.1f}s")


# revision 17
# speedup vs baseline: 1.1978x; 1.1978x over previous
"""MBConv block with MoE routing (depthwise + pointwise expert kernels) on 8 trn2 cores.

Sharding: pure data parallel — batch 64 split 8 samples per core; all weights
replicated. Device kernel computes routing, expert-weight aggregation, expand
conv, per-sample depthwise conv (diagonal-matmul formulation on TensorE),
squeeze-excitation, pointwise projection, BN folds and residual.

Self-contained: hardcodes all shapes; host side only reshapes/prepacks weights.
"""

import os
import sys
import time

for _p in ("/opt/trn_rl_repo", os.path.expanduser("~/.axon_site/_ro/trn_rl_repo")):
    if os.path.isdir(_p) and _p not in sys.path:
        sys.path.insert(0, _p)

import contextlib

import numpy as np

import concourse.bacc as bacc
import concourse.bass as bass
import concourse.tile as tile
from concourse import mybir

F32 = mybir.dt.float32
BF16 = mybir.dt.bfloat16
AF = mybir.ActivationFunctionType
ALU = mybir.AluOpType
AX = mybir.AxisListType

# dims (must match the problem spec)
B, CIN, H, W = 64, 96, 28, 28
NCORES = 8
BL = B // NCORES          # 8 samples per core
E = 4
HID = 576
KK = 5
T = KK * KK               # 25 taps
RED = 24                  # SE reduced dim
RHID = 24                 # routing hidden
COUT = 96
EPS = 1e-3
HW = H * W                # 784
NG = 5                    # ceil(576/128) channel chunks
GP = 128
HIDP = NG * GP            # 640 padded
PW = 32                   # padded row stride
PH = 32                   # padded rows
NH = 2                    # output row halves (14 rows each)
RH = H // NH              # 14
NF = RH * W               # 392 free elems per half


def _build_program(reps=1, ablate=()):
    nc = bacc.Bacc(None, target_bir_lowering=False)

    dt = lambda name, shape: nc.dram_tensor(name, shape, F32, kind="ExternalInput")
    x_d = dt("x", [CIN, BL, HW])
    xbf_d = nc.dram_tensor("xbf", [CIN, BL, HW], BF16, kind="ExternalInput")
    expbf_d = nc.dram_tensor("expbf", [CIN, HIDP], BF16, kind="ExternalInput")
    identbf_d = nc.dram_tensor("identbf", [GP, GP], BF16, kind="ExternalInput")
    expT_d = dt("expT", [CIN, HIDP])
    a1_d = dt("a1", [GP, NG])
    b1_d = dt("b1", [GP, NG])
    a2_d = dt("a2", [GP, NG])
    b2_d = dt("b2", [GP, NG])
    a3_d = dt("a3", [COUT, 1])
    b3_d = dt("b3", [COUT, 1])
    dwT_d = dt("dwT", [GP, E, NG, T])
    pwT_d = dt("pwT", [GP, E, NG, COUT])
    sw1_d = dt("sw1", [GP, NG, RED])
    sw2b_d = dt("sw2b", [RED, NG, GP])
    b2se_d = dt("b2se", [GP, NG])
    rw1_d = dt("rw1", [CIN, RHID])
    rb1_d = dt("rb1", [RHID, 1])
    rw2_d = dt("rw2", [RHID, E])
    rb2_d = dt("rb2", [BL, E])
    sb1_d = dt("sb1", [RED, 1])
    ident_d = dt("ident", [GP, GP])
    y_d = nc.dram_tensor("y", [BL, COUT, HW], F32, kind="ExternalOutput")

    with tile.TileContext(nc) as tc:
        with (
            tc.tile_pool(name="consts", bufs=1) as cp,
            tc.tile_pool(name="dram", bufs=1, space="DRAM") as dp,
            tc.tile_pool(name="xpad", bufs=1) as xpp,
            tc.tile_pool(name="out2", bufs=1) as o2p,
            tc.tile_pool(name="diag", bufs=3) as dgp,
            tc.tile_pool(name="cacc", bufs=2) as accp,
            tc.tile_pool(name="wscp", bufs=2) as wsp,
            tc.tile_pool(name="outb", bufs=2) as obp,
            tc.tile_pool(name="small", bufs=2) as smp,
            tc.tile_pool(name="ppex", bufs=2, space="PSUM") as ppex,
            tc.tile_pool(name="pdw", bufs=2, space="PSUM") as pdwp,
            tc.tile_pool(name="ppw", bufs=1, space="PSUM") as ppwp,
        ):
            # ---- persistent consts ----
            x_sb = cp.tile([CIN, BL, HW], F32, tag="x_sb")
            x_bf = cp.tile([CIN, BL, HW], BF16, tag="x_bf")
            expT = cp.tile([CIN, HIDP], BF16, tag="expT")
            a1 = cp.tile([GP, NG], F32, tag="a1")
            b1 = cp.tile([GP, NG], F32, tag="b1")
            a2 = cp.tile([GP, NG], F32, tag="a2")
            b2 = cp.tile([GP, NG], F32, tag="b2")
            a3 = cp.tile([COUT, 1], F32, tag="a3")
            b3 = cp.tile([COUT, 1], F32, tag="b3")
            dwT = cp.tile([GP, E, NG, T], F32, tag="dwT")
            pwT = cp.tile([GP, E, NG, COUT], F32, tag="pwT")
            sw1 = cp.tile([GP, NG, RED], F32, tag="sw1")
            sw2b = cp.tile([RED, NG, GP], F32, tag="sw2b")
            b2se = cp.tile([GP, NG], F32, tag="b2se")
            rw1 = cp.tile([CIN, RHID], F32, tag="rw1")
            rb1 = cp.tile([RHID, 1], F32, tag="rb1")
            rw2 = cp.tile([RHID, E], F32, tag="rw2")
            rb2 = cp.tile([BL, E], F32, tag="rb2")
            sb1 = cp.tile([RED, 1], F32, tag="sb1")
            ident = cp.tile([GP, GP], BF16, tag="ident")
            kern = cp.tile([GP, NG, BL, T], F32, tag="kern")
            kern_bf = cp.tile([GP, NG, BL, T], BF16, tag="kern_bf")
            pwag = cp.tile([GP, BL, NG, COUT], F32, tag="pwag")
            rw_bc = cp.tile([GP, BL * E], F32, tag="rw_bc")

            # ---- padded x1 tiles: 2 buffers x NG chunks, zeroed once ----
            xp_t = [
                [xpp.tile([GP, 1028], BF16, tag=f"xp{s}g{g}", name=f"xp{s}g{g}")
                 for g in range(NG)]
                for s in range(2)
            ]
            for s in range(2):
                for g in range(NG):
                    nc.gpsimd.memset(xp_t[s][g][:], 0.0)

            out2_t = [o2p.tile([GP, NG, HW], BF16, tag=f"o2{s}", name=f"o2{s}")
                      for s in range(2)]
            s_parts = [cp.tile([GP, NG], F32, tag=f"sp{b}", name=f"sp{b}")
                       for b in range(BL)]
            ident_ap = ident[:]

            def build_diag(eng, dg, b, g):
                # dg[:, t, :] = diag(kern[:, g, b, t]) ; one fused op:
                # out = ident(bcast over taps) * kern(bcast over cols)
                i_b = bass.AP(tensor=ident_ap.tensor, offset=ident_ap.offset,
                              ap=[ident_ap.ap[0], [0, T], [1, GP]])
                k_ap = kern_bf[:, g, b, :]
                k_b = bass.AP(tensor=k_ap.tensor, offset=k_ap.offset,
                              ap=[k_ap.ap[0], k_ap.ap[-1], [0, GP]])
                eng.tensor_mul(dg[:], i_b, k_b)

            def emit_se_pw(b, slot):
                """SE chain + pointwise matmul + bn3/residual + store for sample b."""
                s_sum = s_parts[b]
                pz = ppex.tile([RED, 1], F32, tag="pex", name="pz")
                for g in range(NG):
                    nc.tensor.matmul(pz[:], sw1[:, g], s_sum[:, g : g + 1],
                                     start=(g == 0), stop=(g == NG - 1))
                zt = smp.tile([RED, 1], F32, tag="zt", name="zt")
                nc.scalar.activation(zt[:], pz[:], AF.Silu, bias=sb1[:], scale=1.0)
                psc = ppex.tile([GP, NG], F32, tag="pex", name="psc")
                for g in range(NG):
                    nc.tensor.matmul(psc[:, g : g + 1], sw2b[:, g], zt[:],
                                     start=True, stop=True)
                # sigmoid via tanh (stays in the silu ACT table set):
                # sigmoid(p + b) = 0.5 + 0.5*tanh(0.5*p + 0.5*b); b2se is pre-halved
                sc = smp.tile([GP, NG], F32, tag="sc", name="sc")
                for g in range(NG):
                    nc.scalar.activation(sc[:, g : g + 1], psc[:, g : g + 1],
                                         AF.Tanh, bias=b2se[:, g : g + 1],
                                         scale=0.5)
                nc.vector.tensor_scalar(sc[:], sc[:], 0.5, 0.5,
                                        op0=ALU.mult, op1=ALU.add)
                wsc = wsp.tile([GP, NG, COUT], BF16, tag="wsc", name="wsc")
                for g in range(NG):
                    nc.vector.tensor_scalar_mul(wsc[:, g], pwag[:, b, g],
                                                sc[:, g : g + 1])
                # pointwise projection, accumulate over channel chunks
                po = ppwp.tile([COUT, NH, 512], F32, tag="po", name="po")
                if "nopw" not in ablate:
                    for nh in range(NH):
                        for g in range(NG):
                            nc.tensor.matmul(
                                po[:, nh, :NF], wsc[:, g],
                                out2_t[slot][:, g, nh * NF : (nh + 1) * NF],
                                start=(g == 0), stop=(g == NG - 1))
                ob = obp.tile([COUT, HW], F32, tag="ob", name="ob")
                for nh in range(NH):
                    sl = slice(nh * NF, (nh + 1) * NF)
                    if "nopw" in ablate:
                        nc.vector.tensor_copy(ob[:, sl], x_sb[:, b, sl])
                        continue
                    nc.vector.scalar_tensor_tensor(
                        ob[:, sl], po[:, nh, :NF], a3[:], x_sb[:, b, sl],
                        op0=ALU.mult, op1=ALU.add)
                    nc.vector.tensor_scalar_add(ob[:, sl], ob[:, sl], b3[:])
                nc.sync.dma_start(y_d[b], ob[:])

            def emit_body():
                nc.sync.dma_start(x_sb[:], x_d[:])
                nc.sync.dma_start(x_bf[:], xbf_d[:])
                for t_sb, t_d in [
                    (expT, expbf_d), (a1, a1_d), (b1, b1_d), (a2, a2_d),
                    (b2, b2_d), (a3, a3_d), (b3, b3_d), (dwT, dwT_d),
                    (pwT, pwT_d), (sw1, sw1_d), (sw2b, sw2b_d), (b2se, b2se_d),
                    (rw1, rw1_d), (rb1, rb1_d), (rw2, rw2_d), (rb2, rb2_d),
                    (sb1, sb1_d), (ident, identbf_d),
                ]:
                    nc.sync.dma_start(t_sb[:], t_d[:])

                # routing: pool -> MLP -> softmax (transposed: samples on partitions)
                xsum = cp.tile([CIN, BL], F32, tag="xsum", name="xsum")
                nc.vector.tensor_reduce(xsum[:], x_sb[:], axis=AX.X, op=ALU.add)
                ph1 = ppex.tile([RHID, BL], F32, tag="pex", name="ph1")
                nc.tensor.matmul(ph1[:], rw1[:], xsum[:], start=True, stop=True)
                hdn = cp.tile([RHID, BL], F32, tag="hdn", name="hdn")
                nc.scalar.activation(hdn[:], ph1[:], AF.Relu, bias=rb1[:], scale=1.0)
                pl2 = ppex.tile([BL, E], F32, tag="pex", name="pl2")
                nc.tensor.matmul(pl2[:], hdn[:], rw2[:], start=True, stop=True)
                lt = cp.tile([BL, E], F32, tag="lt", name="lt")
                nc.vector.tensor_add(lt[:], pl2[:], rb2[:])
                mx = cp.tile([BL, 1], F32, tag="mx", name="mx")
                nc.vector.reduce_max(mx[:], lt[:], axis=AX.X)
                nc.vector.tensor_scalar_sub(lt[:], lt[:], mx[:])
                el = cp.tile([BL, E], F32, tag="el", name="el")
                nc.scalar.activation(el[:], lt[:], AF.Exp)
                es = cp.tile([BL, 1], F32, tag="es", name="es")
                nc.vector.reduce_sum(es[:], el[:], axis=AX.X)
                einv = cp.tile([BL, 1], F32, tag="einv", name="einv")
                nc.vector.reciprocal(einv[:], es[:])
                rwT = cp.tile([BL, E], F32, tag="rwT", name="rwT")
                nc.vector.tensor_scalar_mul(rwT[:], el[:], einv[:])
                # broadcast rw to all 128 partitions via DRAM bounce
                rw_dram = dp.tile([BL, E], F32, tag="rwd", name="rwd")
                nc.sync.dma_start(rw_dram[:], rwT[:])
                rwd_ap = rw_dram[:]
                bcast_src = bass.AP(
                    tensor=rwd_ap.tensor, offset=rwd_ap.offset,
                    ap=[[0, GP], [1, BL * E]],
                )
                nc.sync.dma_start(rw_bc[:], bcast_src)

                # expert-weight aggregation (runtime routing weights)
                for b in range(BL):
                    kv = kern[:, :, b, :]
                    pv = pwag[:, b]
                    for e in range(E):
                        s_ap = rw_bc[:, E * b + e : E * b + e + 1]
                        if e == 0:
                            nc.vector.tensor_scalar_mul(kv, dwT[:, e], s_ap)
                            nc.vector.tensor_scalar_mul(pv, pwT[:, e], s_ap)
                        else:
                            nc.vector.scalar_tensor_tensor(
                                kv, dwT[:, e], s_ap, kv, op0=ALU.mult, op1=ALU.add)
                            nc.vector.scalar_tensor_tensor(
                                pv, pwT[:, e], s_ap, pv, op0=ALU.mult, op1=ALU.add)

                nc.vector.tensor_copy(kern_bf[:], kern[:])

                for b in range(BL):
                    slot = b % 2
                    if b > 0:
                        emit_se_pw(b - 1, 1 - slot)
                    # expand conv + bn1 + silu into padded layout
                    for g in range(NG):
                        for nh in range(NH):
                            pex = ppex.tile([GP, NF], F32, tag="pex", name="pex")
                            nc.tensor.matmul(
                                pex[:], expT[:, g * GP : (g + 1) * GP],
                                x_bf[:, b, nh * NF : (nh + 1) * NF],
                                start=True, stop=True)
                            xpv = xp_t[slot][g][:, 68 + nh * 448 : 68 + nh * 448 + 434]
                            xpo = bass.AP(tensor=xpv.tensor, offset=xpv.offset,
                                          ap=[xpv.ap[0], [PW, RH], [1, W]])
                            nc.scalar.activation(
                                xpo, pex[:], AF.Silu,
                                bias=b1[:, g : g + 1], scale=a1[:, g : g + 1])
                    # diagonal weight matrices for this sample
                    dgs = []
                    if "nodiag" not in ablate:
                        for g in range(NG - 1):
                            dg = dgp.tile([GP, T, GP], BF16, tag="dg", name="dg")
                            build_diag(nc.vector if g < 2 else nc.gpsimd, dg, b, g)
                            dgs.append(dg)
                    # depthwise conv: 25 accumulated diag matmuls per chunk/half
                    for g in range(NG - 1):
                        pdw = pdwp.tile([GP, NH, 512], F32, tag="pdw", name="pdw")
                        taps = [] if "noconv" in ablate else list(range(T))
                        if "conv1tap" in ablate:
                            taps = [12]
                        for nh in range(NH):
                            for ti, t in enumerate(taps):
                                kh, kw = divmod(t, KK)
                                lhs = (ident[:, :] if "nodiag" in ablate
                                       else dgs[g][:, t, :])
                                off = 448 * nh + 32 * kh + kw
                                rhs = xp_t[slot][g][:, off : off + 448]
                                nc.tensor.matmul(pdw[:, nh, :448], lhs, rhs,
                                                 start=(ti == 0),
                                                 stop=(ti == len(taps) - 1))
                        # bn2 + silu over both banks in one op (+ SE spatial sum)
                        pv = pdw[:, 0, 2:3]
                        pvo = bass.AP(tensor=pv.tensor, offset=pv.offset,
                                      ap=[pv.ap[0], [512, NH], [PW, RH], [1, W]])
                        nc.scalar.activation(
                            out2_t[slot][:, g, :], pvo, AF.Silu,
                            bias=b2[:, g : g + 1], scale=a2[:, g : g + 1],
                            accum_out=s_parts[b][:, g : g + 1])
                    # ---- g=4 chunk on DVE: bf16 scalar*tensor+acc tap chain ----
                    g4 = NG - 1
                    acc = accp.tile([GP, 896], BF16, tag="acc", name="acc")
                    taps4 = [12] if "conv1tap" in ablate else list(range(T))
                    for ti, t in enumerate(taps4):
                        kh, kw = divmod(t, KK)
                        xin = xp_t[slot][g4][:, 32 * kh + kw : 32 * kh + kw + 896]
                        k_ap = kern[:, g4, b, t : t + 1]
                        if ti == 0:
                            nc.vector.tensor_scalar_mul(acc[:], xin, k_ap)
                        else:
                            nc.vector.scalar_tensor_tensor(
                                acc[:], xin, k_ap, acc[:],
                                op0=ALU.mult, op1=ALU.add)
                    av = acc[:, 2:868]
                    avo = bass.AP(tensor=av.tensor, offset=av.offset,
                                  ap=[av.ap[0], [PW, H], [1, W]])
                    nc.scalar.activation(
                        out2_t[slot][:, g4, :], avo, AF.Silu,
                        bias=b2[:, g4 : g4 + 1], scale=a2[:, g4 : g4 + 1],
                        accum_out=s_parts[b][:, 4:5])
                emit_se_pw(BL - 1, (BL - 1) % 2)

            loop_ctx = (tc.For_i(0, reps, 1, hint_engines=(mybir.EngineType.PE,))
                        if reps > 1 else contextlib.nullcontext())
            with loop_ctx:
                emit_body()

    nc.compile()
    return nc


_NC = None


def _get_nc():
    global _NC
    if _NC is None:
        _NC = _build_program()
    return _NC


def _prep_maps(x, r_w1, r_b1, r_w2, r_b2, exp_w,
               bn1_g, bn1_b, bn1_m, bn1_v, dw_w,
               bn2_g, bn2_b, bn2_m, bn2_v,
               se_w1, se_b1, se_w2, se_b2, pw_w,
               bn3_g, bn3_b, bn3_m, bn3_v):
    f = np.float32
    bfdt = mybir.dt.np(BF16)
    x = np.asarray(x, f).reshape(B, CIN, HW)

    def fold_bn(g, bvec, m, v):
        a = np.asarray(g, f) / np.sqrt(np.asarray(v, f) + EPS)
        return a, np.asarray(bvec, f) - np.asarray(m, f) * a

    a1v, b1v = fold_bn(bn1_g, bn1_b, bn1_m, bn1_v)
    a2v, b2v = fold_bn(bn2_g, bn2_b, bn2_m, bn2_v)
    a3v, b3v = fold_bn(bn3_g, bn3_b, bn3_m, bn3_v)

    def chunk(v):  # [HID] -> [GP, NG] padded
        vp = np.concatenate([np.asarray(v, f), np.zeros(HIDP - HID, f)])
        return vp.reshape(NG, GP).T.copy()

    expT = np.zeros((CIN, HIDP), f)
    expT[:, :HID] = np.asarray(exp_w, f).T
    dwf = np.asarray(dw_w, f).reshape(E, HID, T)
    dwT = np.zeros((GP, E, NG, T), f)
    pwT = np.zeros((GP, E, NG, COUT), f)
    sw1 = np.zeros((GP, NG, RED), f)
    sw2b = np.zeros((RED, NG, GP), f)
    b2se = np.zeros((GP, NG), f)
    for g in range(NG):
        n = min(GP, HID - g * GP)
        cs = slice(g * GP, g * GP + n)
        dwT[:n, :, g, :] = dwf[:, cs, :].transpose(1, 0, 2)
        pwT[:n, :, g, :] = np.asarray(pw_w, f)[:, :, cs].transpose(2, 0, 1)
        sw1[:n, g, :] = (np.asarray(se_w1, f)[:, cs] / HW).T
        sw2b[:, g, :n] = np.asarray(se_w2, f)[cs, :].T
        b2se[:n, g] = np.asarray(se_b2, f)[cs] / 2

    common = dict(
        expT=expT, a1=chunk(a1v), b1=chunk(b1v), a2=chunk(a2v), b2=chunk(b2v),
        a3=a3v.reshape(COUT, 1), b3=b3v.reshape(COUT, 1),
        dwT=dwT, pwT=pwT, sw1=sw1, sw2b=sw2b, b2se=b2se,
        rw1=(np.asarray(r_w1, f).T / HW).copy(),
        rb1=np.asarray(r_b1, f).reshape(RHID, 1),
        rw2=np.asarray(r_w2, f).T.copy(),
        rb2=np.tile(np.asarray(r_b2, f), (BL, 1)),
        sb1=np.asarray(se_b1, f).reshape(RED, 1),
        ident=np.eye(GP, dtype=f),
        expbf=expT.astype(bfdt),
        identbf=np.eye(GP, dtype=bfdt),
    )
    out = []
    for c in range(NCORES):
        xs = np.ascontiguousarray(x[c * BL : (c + 1) * BL].transpose(1, 0, 2))
        out.append(dict(common, x=xs, xbf=xs.astype(bfdt)))
    return out


def kernel(**inputs):
    from concourse.bass_utils import run_bass_kernel_spmd

    nc = _get_nc()
    in_maps = _prep_maps(**inputs)
    res = run_bass_kernel_spmd(nc, in_maps, core_ids=list(range(NCORES)))
    y = np.concatenate([res.results[c]["y"] for c in range(NCORES)], axis=0)
    return y.reshape(B, COUT, H, W).astype(np.float32)


if __name__ == "__main__":
    t0 = time.time()
    nc = _get_nc()
    print(f"build+compile: {time.time()-t0:.1f}s")
